# revision 11
# baseline (speedup 1.0000x reference)
"""Trainium2 Bass kernel for nn_Loss_2 (weighted BCE + index-gathered CE mean).

Data-parallel over 8 NeuronCores: each core processes 8 of the 64 batches
(131072 tokens = [P=128, T=1024]), computes per-partition partial sums
on-chip; host sums 8x[128,1] partials and divides by B*S.

Pipeline: all comb chunk DMAs issued upfront; per chunk
    nem     = (iota_class != idxg)       broadcast-3D TT, bf16, 2x DVE
    masked1 = max(nem, comb)             comb[t,c] at the gathered class,
                                         1.0 elsewhere
    cce_k   = accum(Ln(masked1))         ScalarE, ln(1)=0 for non-matches
Chunk sizes decrease at the end so the last Ln (which gates the tail) is
short.  ys==0 tokens have idxg pushed out of [0,20) (sentinel +64): no
class matches, contribution ln(1)=0.  BCE by the same select-by-max trick:
    m1 = max(1-ys, ps)  -> accum(Ln)*W1 ; m0 = max(ys, 1-ps) -> accum(Ln)*W0
total = -W1*a1 - W0*a0 - sum(cce accums)
"""

import sys

if '/opt/trn_rl_repo' not in sys.path:
    sys.path.insert(0, '/opt/trn_rl_repo')

import numpy as np
import ml_dtypes

import concourse.bass as bass
import concourse.bacc as bacc
import concourse.tile as tile
import concourse.mybir as mybir
from concourse.bass_utils import run_bass_kernel_spmd

F32 = mybir.dt.float32
BF16 = mybir.dt.bfloat16

B, S, C = 64, 16384, 20
W0, W1 = 0.51, 19.05
BIG = 64.0
P = 128
N_CORES = 8
T = (B // N_CORES) * S // P        # 1024 tokens per partition per core
BLK = 4                            # tokens per block (packed last dim)
CHUNK_TOK = [256, 352, 352, 64]    # tokens/partition per chunk (sum = T)
NC = len(CHUNK_TOK)
CHUNK_OFF = [sum(CHUNK_TOK[:i]) for i in range(NC)]

NT = NC                            # kept for test.py cache-key compat
Tp = T


def _build(NC_, T_):
    nc = bacc.Bacc("TRN2", target_bir_lowering=False, debug=False)

    comb_d = nc.dram_tensor("comb", [P, T_ * C], BF16, kind="ExternalInput").ap()
    idxg_d = nc.dram_tensor("idxg", [P, T_], BF16, kind="ExternalInput").ap()
    ps_d = nc.dram_tensor("ps", [P, T_], BF16, kind="ExternalInput").ap()
    out_d = nc.dram_tensor("out", [P, 1], F32, kind="ExternalOutput").ap()

    ne = mybir.AluOpType.not_equal
    mx = mybir.AluOpType.max
    Ln = mybir.ActivationFunctionType.Ln

    with tile.TileContext(nc) as tc:
        with (
            tc.tile_pool(name="const", bufs=1) as const_pool,
            tc.tile_pool(name="comb", bufs=1) as comb_pool,
            tc.tile_pool(name="nem", bufs=2) as nem_pool,
            tc.tile_pool(name="msk", bufs=2) as msk_pool,
            tc.tile_pool(name="small", bufs=1) as small_pool,
        ):
            parts = const_pool.tile([P, NC_ + 2], F32)

            # idxg first so the cce chain starts ASAP; ps separately
            idxg_t = small_pool.tile([P, T_], BF16, tag="idxg")
            nc.sync.dma_start(idxg_t[:], idxg_d[:])
            ps_t = small_pool.tile([P, T_], BF16, tag="ps")
            nc.sync.dma_start(ps_t[:], ps_d[:])
            idxg = idxg_t[:]
            ps = ps_t[:]

            comb_ts = []
            for k in range(NC_):
                fk = CHUNK_TOK[k] * C
                off = CHUNK_OFF[k] * C
                comb_t = comb_pool.tile([P, fk], BF16, tag=f"comb{k}")
                nc.sync.dma_start(comb_t[:], comb_d[:, off:off + fk])
                comb_ts.append(comb_t)

            iota_cb = const_pool.tile([P, C * BLK], BF16)
            nc.gpsimd.iota(iota_cb[:], pattern=[[1, C], [0, BLK]], base=0,
                           channel_multiplier=0,
                           allow_small_or_imprecise_dtypes=True)
            iota4 = iota_cb[:].rearrange("p (o c b) -> p o c b", o=1, c=C, b=BLK)

            # CCE chunks
            for k in range(NC_):
                tk = CHUNK_TOK[k]
                nbk = tk // BLK
                fk = tk * C
                idx_k = idxg[:, CHUNK_OFF[k]:CHUNK_OFF[k] + tk]
                idx4 = idx_k.rearrange("p (n o b) -> p n o b", o=1, b=BLK)
                b_iota, b_idx = bass.broadcast_tensor_aps(iota4, idx4)

                nem = nem_pool.tile([P, fk], BF16, tag="nem")
                nem_v = nem[:].rearrange("p (n c b) -> p n c b", c=C, b=BLK)
                nc.vector.tensor_tensor(nem_v, b_iota, b_idx, ne)

                msk = msk_pool.tile([P, fk], BF16, tag="msk")
                nc.vector.tensor_tensor(msk[:], nem[:], comb_ts[k][:], mx)

                # Ln output overwrites nem (dead after msk)
                nc.scalar.activation(nem[:], msk[:], Ln,
                                     accum_out=parts[:, k:k + 1])

            # BCE (after cce so it doesn't delay the pipeline)
            ys = small_pool.tile([P, T_], BF16, tag="ys")
            nc.vector.tensor_scalar(ys[:], idxg, 32.0, None, mybir.AluOpType.is_lt)
            ys1m = small_pool.tile([P, T_], BF16, tag="ys1m")
            nc.vector.tensor_scalar(ys1m[:], idxg, 32.0, None, mybir.AluOpType.is_ge)
            ps1m = small_pool.tile([P, T_], BF16, tag="ps1m")
            nc.vector.tensor_scalar(ps1m[:], ps, -1.0, 1.0,
                                    mybir.AluOpType.mult, mybir.AluOpType.add)
            m1 = small_pool.tile([P, T_], BF16, tag="m1")
            nc.vector.tensor_tensor(m1[:], ys1m[:], ps, mx)
            m0 = small_pool.tile([P, T_], BF16, tag="m0")
            nc.vector.tensor_tensor(m0[:], ys[:], ps1m[:], mx)
            j1 = small_pool.tile([P, T_], BF16, tag="j1")
            nc.scalar.activation(j1[:], m1[:], Ln, accum_out=parts[:, NC_:NC_ + 1])
            j0 = small_pool.tile([P, T_], BF16, tag="j0")
            nc.scalar.activation(j0[:], m0[:], Ln,
                                 accum_out=parts[:, NC_ + 1:NC_ + 2])

            # total = -W1*a1 - W0*a0 - sum(cce parts)
            rC = const_pool.tile([P, 1], F32)
            nc.vector.tensor_reduce(rC[:], parts[:, 0:NC_], axis=mybir.AxisListType.X,
                                    op=mybir.AluOpType.add)
            wA = const_pool.tile([P, 1], F32)
            nc.vector.tensor_scalar(wA[:], parts[:, NC_:NC_ + 1], -W1, None,
                                    mybir.AluOpType.mult)
            total = const_pool.tile([P, 1], F32)
            nc.vector.scalar_tensor_tensor(
                total[:], parts[:, NC_ + 1:NC_ + 2], -W0, wA[:],
                op0=mybir.AluOpType.mult, op1=mybir.AluOpType.add)
            nc.vector.tensor_tensor(total[:], total[:], rC[:],
                                    mybir.AluOpType.subtract)

            nc.sync.dma_start(out_d[:], total[:])

    nc.compile()
    return nc


_NC_CACHE = {}


def make_in_maps(y_pred_stroke, y_pred_comb, y_stroke, y_comb):
    y_pred_stroke = np.asarray(y_pred_stroke, dtype=np.float32)
    y_pred_comb = np.asarray(y_pred_comb, dtype=np.float32)
    y_stroke = np.asarray(y_stroke, dtype=np.float32)
    y_comb = np.asarray(y_comb)
    Bc = B // N_CORES
    in_maps = []
    for core in range(N_CORES):
        sl = slice(core * Bc, (core + 1) * Bc)
        # block-token layout over full T: [P, T, C] -> [P, T/BLK, C, BLK]
        cm = np.ascontiguousarray(y_pred_comb[sl]).reshape(P, T, C)
        cm = cm.reshape(P, T // BLK, BLK, C).transpose(0, 1, 3, 2)
        comb = np.ascontiguousarray(cm).reshape(P, T * C)
        comb = comb.astype(ml_dtypes.bfloat16)

        idxg = (np.ascontiguousarray(y_comb[sl]).astype(np.float32)
                + (1.0 - np.ascontiguousarray(y_stroke[sl])[..., 0]) * BIG
                ).reshape(P, T).astype(ml_dtypes.bfloat16)
        ps = (np.ascontiguousarray(y_pred_stroke[sl])
              .reshape(P, T).astype(ml_dtypes.bfloat16))
        in_maps.append({"comb": comb, "idxg": idxg, "ps": ps})
    return in_maps


def kernel(y_pred_stroke, y_pred_comb, y_stroke, y_comb):
    key = (NT, Tp)
    if key not in _NC_CACHE:
        _NC_CACHE[key] = _build(NC, T)
    nc = _NC_CACHE[key]
    in_maps = make_in_maps(y_pred_stroke, y_pred_comb, y_stroke, y_comb)
    res = run_bass_kernel_spmd(nc, in_maps, list(range(N_CORES)))
    total = 0.0
    for r in res.results:
        total += r["out"].astype(np.float64).sum()
    return np.asarray([total / (B * S)], dtype=np.float32)


# revision 12
# speedup vs baseline: 1.1789x; 1.1789x over previous
"""Trainium2 Bass kernel for nn_Loss_2 (weighted BCE + index-gathered CE mean).

Data-parallel over 8 NeuronCores: each core processes 8 of the 64 batches
(131072 tokens = [P=128, T=1024]), computes per-partition partial sums
on-chip; host sums 8x[128,1] partials and divides by B*S.

Pipeline: all comb chunk DMAs issued upfront; per chunk
    nem     = (iota_class != idxg)       broadcast-3D TT, bf16, 2x DVE
    masked1 = max(nem, comb)             comb[t,c] at the gathered class,
                                         1.0 elsewhere
    cce_k   = accum(Ln(masked1))         ScalarE, ln(1)=0 for non-matches
ys==0 tokens have idxg pushed out of [0,20) (sentinel +64): no class
matches, contribution ln(1)=0.  BCE by the same select-by-max trick:
    m1 = max(1-ys, ps)  -> accum(Ln)*W1 ; m0 = max(ys, 1-ps) -> accum(Ln)*W0
total = -W1*a1 - W0*a0 - sum(cce accums)
"""

import sys

if '/opt/trn_rl_repo' not in sys.path:
    sys.path.insert(0, '/opt/trn_rl_repo')

import numpy as np
import ml_dtypes

import concourse.bass as bass
import concourse.bacc as bacc
import concourse.tile as tile
import concourse.mybir as mybir
from concourse.bass_utils import run_bass_kernel_spmd

F32 = mybir.dt.float32
BF16 = mybir.dt.bfloat16

B, S, C = 64, 16384, 20
W0, W1 = 0.51, 19.05
BIG = 64.0
P = 128
N_CORES = 8
T = (B // N_CORES) * S // P        # 1024 tokens per partition per core
BLK = 4                            # tokens per block (packed last dim)
NC = 4                             # comb chunks for pipelining
TC = T // NC                       # tokens per partition per chunk
NBLK = TC // BLK                   # blocks per chunk
FREE_C = NBLK * C * BLK            # comb elements per partition per chunk

NT = NC                            # kept for test.py cache-key compat
Tp = T


def _build(NC_, T_):
    nc = bacc.Bacc("TRN2", target_bir_lowering=False, debug=False)

    comb_d = nc.dram_tensor("comb", [NC_, P, FREE_C], BF16, kind="ExternalInput").ap()
    idxg_d = nc.dram_tensor("idxg", [P, T_], BF16, kind="ExternalInput").ap()
    ps_d = nc.dram_tensor("ps", [P, T_], BF16, kind="ExternalInput").ap()
    out_d = nc.dram_tensor("out", [P, 1], F32, kind="ExternalOutput").ap()

    ne = mybir.AluOpType.not_equal
    mx = mybir.AluOpType.max
    Ln = mybir.ActivationFunctionType.Ln

    with tile.TileContext(nc) as tc:
        with (
            tc.tile_pool(name="const", bufs=1) as const_pool,
            tc.tile_pool(name="comb", bufs=NC_) as comb_pool,
            tc.tile_pool(name="nem", bufs=2) as nem_pool,
            tc.tile_pool(name="msk", bufs=2) as msk_pool,
            tc.tile_pool(name="small", bufs=1) as small_pool,
        ):
            parts = const_pool.tile([P, NC_ + 2], F32)

            # idxg first so the cce chain starts ASAP; ps separately
            idxg_t = small_pool.tile([P, T_], BF16, tag="idxg")
            nc.sync.dma_start(idxg_t[:], idxg_d[:])
            ps_t = small_pool.tile([P, T_], BF16, tag="ps")
            nc.sync.dma_start(ps_t[:], ps_d[:])
            idxg = idxg_t[:]
            ps = ps_t[:]

            comb_ts = []
            for k in range(NC_):
                comb_t = comb_pool.tile([P, FREE_C], BF16, tag="comb")
                nc.sync.dma_start(comb_t[:], comb_d[k])
                comb_ts.append(comb_t)

            iota_cb = const_pool.tile([P, C * BLK], BF16)
            nc.gpsimd.iota(iota_cb[:], pattern=[[1, C], [0, BLK]], base=0,
                           channel_multiplier=0,
                           allow_small_or_imprecise_dtypes=True)
            iota4 = iota_cb[:].rearrange("p (o c b) -> p o c b", o=1, c=C, b=BLK)

            # CCE chunks
            for k in range(NC_):
                idx_k = idxg[:, k * TC:(k + 1) * TC]
                idx4 = idx_k.rearrange("p (n o b) -> p n o b", o=1, b=BLK)
                b_iota, b_idx = bass.broadcast_tensor_aps(iota4, idx4)

                nem = nem_pool.tile([P, FREE_C], BF16, tag="nem")
                nem_v = nem[:].rearrange("p (n c b) -> p n c b", c=C, b=BLK)
                nc.vector.tensor_tensor(nem_v, b_iota, b_idx, ne)

                msk = msk_pool.tile([P, FREE_C], BF16, tag="msk")
                nc.vector.tensor_tensor(msk[:], nem[:], comb_ts[k][:], mx)

                # Ln output overwrites nem (dead after msk)
                nc.scalar.activation(nem[:], msk[:], Ln,
                                     accum_out=parts[:, k:k + 1])

            # BCE (after cce so it doesn't delay the pipeline)
            ys = small_pool.tile([P, T_], BF16, tag="ys")
            nc.vector.tensor_scalar(ys[:], idxg, 32.0, None, mybir.AluOpType.is_lt)
            ys1m = small_pool.tile([P, T_], BF16, tag="ys1m")
            nc.vector.tensor_scalar(ys1m[:], idxg, 32.0, None, mybir.AluOpType.is_ge)
            ps1m = small_pool.tile([P, T_], BF16, tag="ps1m")
            nc.vector.tensor_scalar(ps1m[:], ps, -1.0, 1.0,
                                    mybir.AluOpType.mult, mybir.AluOpType.add)
            m1 = small_pool.tile([P, T_], BF16, tag="m1")
            nc.vector.tensor_tensor(m1[:], ys1m[:], ps, mx)
            m0 = small_pool.tile([P, T_], BF16, tag="m0")
            nc.vector.tensor_tensor(m0[:], ys[:], ps1m[:], mx)
            j1 = small_pool.tile([P, T_], BF16, tag="j1")
            nc.scalar.activation(j1[:], m1[:], Ln, accum_out=parts[:, NC_:NC_ + 1])
            j0 = small_pool.tile([P, T_], BF16, tag="j0")
            nc.scalar.activation(j0[:], m0[:], Ln,
                                 accum_out=parts[:, NC_ + 1:NC_ + 2])

            # total = -W1*a1 - W0*a0 - sum(cce parts)
            rC = const_pool.tile([P, 1], F32)
            nc.vector.tensor_reduce(rC[:], parts[:, 0:NC_], axis=mybir.AxisListType.X,
                                    op=mybir.AluOpType.add)
            wA = const_pool.tile([P, 1], F32)
            nc.vector.tensor_scalar(wA[:], parts[:, NC_:NC_ + 1], -W1, None,
                                    mybir.AluOpType.mult)
            total = const_pool.tile([P, 1], F32)
            nc.vector.scalar_tensor_tensor(
                total[:], parts[:, NC_ + 1:NC_ + 2], -W0, wA[:],
                op0=mybir.AluOpType.mult, op1=mybir.AluOpType.add)
            nc.vector.tensor_tensor(total[:], total[:], rC[:],
                                    mybir.AluOpType.subtract)

            nc.sync.dma_start(out_d[:], total[:])

    nc.compile()
    return nc


_NC_CACHE = {}


def make_in_maps(y_pred_stroke, y_pred_comb, y_stroke, y_comb):
    y_pred_stroke = np.asarray(y_pred_stroke, dtype=np.float32)
    y_pred_comb = np.asarray(y_pred_comb, dtype=np.float32)
    y_stroke = np.asarray(y_stroke, dtype=np.float32)
    y_comb = np.asarray(y_comb)
    Bc = B // N_CORES
    in_maps = []
    for core in range(N_CORES):
        sl = slice(core * Bc, (core + 1) * Bc)
        # block-token layout: [P, T, C] -> [NC, P, nblk, C, BLK]
        cm = np.ascontiguousarray(y_pred_comb[sl]).reshape(P, T, C)
        cm = cm.reshape(P, NC, NBLK, BLK, C).transpose(1, 0, 2, 4, 3)
        comb = np.ascontiguousarray(cm).reshape(NC, P, FREE_C)
        comb = comb.astype(ml_dtypes.bfloat16)

        idxg = (np.ascontiguousarray(y_comb[sl]).astype(np.float32)
                + (1.0 - np.ascontiguousarray(y_stroke[sl])[..., 0]) * BIG
                ).reshape(P, T).astype(ml_dtypes.bfloat16)
        ps = (np.ascontiguousarray(y_pred_stroke[sl])
              .reshape(P, T).astype(ml_dtypes.bfloat16))
        in_maps.append({"comb": comb, "idxg": idxg, "ps": ps})
    return in_maps


def kernel(y_pred_stroke, y_pred_comb, y_stroke, y_comb):
    key = (NT, Tp)
    if key not in _NC_CACHE:
        _NC_CACHE[key] = _build(NC, T)
    nc = _NC_CACHE[key]
    in_maps = make_in_maps(y_pred_stroke, y_pred_comb, y_stroke, y_comb)
    res = run_bass_kernel_spmd(nc, in_maps, list(range(N_CORES)))
    total = 0.0
    for r in res.results:
        total += r["out"].astype(np.float64).sum()
    return np.asarray([total / (B * S)], dtype=np.float32)


# revision 13
# speedup vs baseline: 1.1848x; 1.0050x over previous
"""Trainium2 Bass kernel for nn_Loss_2 (weighted BCE + index-gathered CE mean).

Data-parallel over 8 NeuronCores: each core processes 8 of the 64 batches
(131072 tokens = [P=128, T=1024]), computes per-partition partial sums
on-chip; host sums 8x[128,1] partials and divides by B*S.

Pipeline: all comb chunk DMAs issued upfront; per chunk
    nem     = (iota_class != idxg)       broadcast-3D TT, bf16, 2x DVE
    masked1 = max(nem, comb)             comb[t,c] at the gathered class,
                                         1.0 elsewhere
    cce_k   = accum(Ln(masked1))         ScalarE, ln(1)=0 for non-matches
ys==0 tokens have idxg pushed out of [0,20) (sentinel +64): no class
matches, contribution ln(1)=0.  BCE by the same select-by-max trick:
    m1 = max(1-ys, ps)  -> accum(Ln)*W1 ; m0 = max(ys, 1-ps) -> accum(Ln)*W0
total = -W1*a1 - W0*a0 - sum(cce accums)
"""

import sys

if '/opt/trn_rl_repo' not in sys.path:
    sys.path.insert(0, '/opt/trn_rl_repo')

import numpy as np
import ml_dtypes

import concourse.bass as bass
import concourse.bacc as bacc
import concourse.tile as tile
import concourse.mybir as mybir
from concourse.bass_utils import run_bass_kernel_spmd

F32 = mybir.dt.float32
BF16 = mybir.dt.bfloat16

B, S, C = 64, 16384, 20
W0, W1 = 0.51, 19.05
BIG = 64.0
P = 128
N_CORES = 8
T = (B // N_CORES) * S // P        # 1024 tokens per partition per core
BLK = 4                            # tokens per block (packed last dim)
NC = 8                             # comb chunks for pipelining
TC = T // NC                       # tokens per partition per chunk
NBLK = TC // BLK                   # blocks per chunk
FREE_C = NBLK * C * BLK            # comb elements per partition per chunk

NT = NC                            # kept for test.py cache-key compat
Tp = T


def _build(NC_, T_):
    nc = bacc.Bacc("TRN2", target_bir_lowering=False, debug=False)

    comb_d = nc.dram_tensor("comb", [NC_, P, FREE_C], BF16, kind="ExternalInput").ap()
    idxg_d = nc.dram_tensor("idxg", [P, T_], BF16, kind="ExternalInput").ap()
    ps_d = nc.dram_tensor("ps", [P, T_], BF16, kind="ExternalInput").ap()
    out_d = nc.dram_tensor("out", [P, 1], F32, kind="ExternalOutput").ap()

    ne = mybir.AluOpType.not_equal
    mx = mybir.AluOpType.max
    Ln = mybir.ActivationFunctionType.Ln

    with tile.TileContext(nc) as tc:
        with (
            tc.tile_pool(name="const", bufs=1) as const_pool,
            tc.tile_pool(name="comb", bufs=NC_) as comb_pool,
            tc.tile_pool(name="nem", bufs=2) as nem_pool,
            tc.tile_pool(name="msk", bufs=2) as msk_pool,
            tc.tile_pool(name="small", bufs=1) as small_pool,
        ):
            parts = const_pool.tile([P, NC_ + 2], F32)

            # idxg first so the cce chain starts ASAP; ps separately
            idxg_t = small_pool.tile([P, T_], BF16, tag="idxg")
            nc.sync.dma_start(idxg_t[:], idxg_d[:])
            ps_t = small_pool.tile([P, T_], BF16, tag="ps")
            nc.sync.dma_start(ps_t[:], ps_d[:])
            idxg = idxg_t[:]
            ps = ps_t[:]

            comb_ts = []
            for k in range(NC_):
                comb_t = comb_pool.tile([P, FREE_C], BF16, tag="comb")
                nc.sync.dma_start(comb_t[:], comb_d[k])
                comb_ts.append(comb_t)

            iota_cb = const_pool.tile([P, C * BLK], BF16)
            nc.gpsimd.iota(iota_cb[:], pattern=[[1, C], [0, BLK]], base=0,
                           channel_multiplier=0,
                           allow_small_or_imprecise_dtypes=True)
            iota4 = iota_cb[:].rearrange("p (o c b) -> p o c b", o=1, c=C, b=BLK)

            # CCE chunks
            for k in range(NC_):
                idx_k = idxg[:, k * TC:(k + 1) * TC]
                idx4 = idx_k.rearrange("p (n o b) -> p n o b", o=1, b=BLK)
                b_iota, b_idx = bass.broadcast_tensor_aps(iota4, idx4)

                nem = nem_pool.tile([P, FREE_C], BF16, tag="nem")
                nem_v = nem[:].rearrange("p (n c b) -> p n c b", c=C, b=BLK)
                nc.vector.tensor_tensor(nem_v, b_iota, b_idx, ne)

                msk = msk_pool.tile([P, FREE_C], BF16, tag="msk")
                nc.vector.tensor_tensor(msk[:], nem[:], comb_ts[k][:], mx)

                # Ln output overwrites nem (dead after msk)
                nc.scalar.activation(nem[:], msk[:], Ln,
                                     accum_out=parts[:, k:k + 1])

            # BCE (after cce so it doesn't delay the pipeline)
            ys = small_pool.tile([P, T_], BF16, tag="ys")
            nc.vector.tensor_scalar(ys[:], idxg, 32.0, None, mybir.AluOpType.is_lt)
            ys1m = small_pool.tile([P, T_], BF16, tag="ys1m")
            nc.vector.tensor_scalar(ys1m[:], idxg, 32.0, None, mybir.AluOpType.is_ge)
            ps1m = small_pool.tile([P, T_], BF16, tag="ps1m")
            nc.vector.tensor_scalar(ps1m[:], ps, -1.0, 1.0,
                                    mybir.AluOpType.mult, mybir.AluOpType.add)
            m1 = small_pool.tile([P, T_], BF16, tag="m1")
            nc.vector.tensor_tensor(m1[:], ys1m[:], ps, mx)
            m0 = small_pool.tile([P, T_], BF16, tag="m0")
            nc.vector.tensor_tensor(m0[:], ys[:], ps1m[:], mx)
            j1 = small_pool.tile([P, T_], BF16, tag="j1")
            nc.scalar.activation(j1[:], m1[:], Ln, accum_out=parts[:, NC_:NC_ + 1])
            j0 = small_pool.tile([P, T_], BF16, tag="j0")
            nc.scalar.activation(j0[:], m0[:], Ln,
                                 accum_out=parts[:, NC_ + 1:NC_ + 2])

            # total = -W1*a1 - W0*a0 - sum(cce parts)
            rC = const_pool.tile([P, 1], F32)
            nc.vector.tensor_reduce(rC[:], parts[:, 0:NC_], axis=mybir.AxisListType.X,
                                    op=mybir.AluOpType.add)
            wA = const_pool.tile([P, 1], F32)
            nc.vector.tensor_scalar(wA[:], parts[:, NC_:NC_ + 1], -W1, None,
                                    mybir.AluOpType.mult)
            total = const_pool.tile([P, 1], F32)
            nc.vector.scalar_tensor_tensor(
                total[:], parts[:, NC_ + 1:NC_ + 2], -W0, wA[:],
                op0=mybir.AluOpType.mult, op1=mybir.AluOpType.add)
            nc.vector.tensor_tensor(total[:], total[:], rC[:],
                                    mybir.AluOpType.subtract)

            nc.sync.dma_start(out_d[:], total[:])

    nc.compile()
    return nc


_NC_CACHE = {}


def make_in_maps(y_pred_stroke, y_pred_comb, y_stroke, y_comb):
    y_pred_stroke = np.asarray(y_pred_stroke, dtype=np.float32)
    y_pred_comb = np.asarray(y_pred_comb, dtype=np.float32)
    y_stroke = np.asarray(y_stroke, dtype=np.float32)
    y_comb = np.asarray(y_comb)
    Bc = B // N_CORES
    in_maps = []
    for core in range(N_CORES):
        sl = slice(core * Bc, (core + 1) * Bc)
        # block-token layout: [P, T, C] -> [NC, P, nblk, C, BLK]
        cm = np.ascontiguousarray(y_pred_comb[sl]).reshape(P, T, C)
        cm = cm.reshape(P, NC, NBLK, BLK, C).transpose(1, 0, 2, 4, 3)
        comb = np.ascontiguousarray(cm).reshape(NC, P, FREE_C)
        comb = comb.astype(ml_dtypes.bfloat16)

        idxg = (np.ascontiguousarray(y_comb[sl]).astype(np.float32)
                + (1.0 - np.ascontiguousarray(y_stroke[sl])[..., 0]) * BIG
                ).reshape(P, T).astype(ml_dtypes.bfloat16)
        ps = (np.ascontiguousarray(y_pred_stroke[sl])
              .reshape(P, T).astype(ml_dtypes.bfloat16))
        in_maps.append({"comb": comb, "idxg": idxg, "ps": ps})
    return in_maps


def kernel(y_pred_stroke, y_pred_comb, y_stroke, y_comb):
    key = (NT, Tp)
    if key not in _NC_CACHE:
        _NC_CACHE[key] = _build(NC, T)
    nc = _NC_CACHE[key]
    in_maps = make_in_maps(y_pred_stroke, y_pred_comb, y_stroke, y_comb)
    res = run_bass_kernel_spmd(nc, in_maps, list(range(N_CORES)))
    total = 0.0
    for r in res.results:
        total += r["out"].astype(np.float64).sum()
    return np.asarray([total / (B * S)], dtype=np.float32)
